# revision 1
# baseline (speedup 1.0000x reference)
"""Trainium2 Bass kernel for ContrastMemoryBankCELoss.

Strategy (8 NeuronCores, SPMD, no collectives):
  * The 2048 anchor rows (8 views x 256 anchors, view-major) are sorted by
    class label on the host and sharded 256 rows/core (data parallel).
  * The queue (classes 1..18, 36864 contrast vectors) is replicated to every
    core, staged transposed+tiled in bf16: qt[c, k, 128, 2048].
  * Per core, per 128-row group g and class block c: PE computes the raw dot
    block z = at_g^T @ qt_c in PSUM (f32 accum), ScalarE computes
    exp(10*z) with accum_out giving the per-row block sum Tbuf[:, c].
  * The softmax loss is shift-invariant, so no row-max pass is needed
    (|dot| <= 1 for normalized vectors -> exp(10 z) <= e^10, f32-safe).
  * Per-row positive-block statistics are recovered without any gather:
      B_r   = <Tbuf[r, :], onehot_r>          (own-block exp sum)
      zbs_r = dot(anchor_r, sum of own block) (via host-gathered per-row
              block-sum vectors + diagonal-of-matmul extraction)
      zd_r  = dot(anchor_r, queue[1][orig_r]) (diagonal self-contrast term,
              only active for label-1 rows)
  * Positive log-prob tail uses ln(exp(a)+S) = ln S + exp(a)/S to first
    order (max exp(a)/S ~ 2e-3 for this regime; validated to ~2e-7 final
    relative error against the exact reference).
  * Per-row losses DMA back; host sums / 2048. All per-core differences are
    data-only (host-staged tensors), so one program serves all 8 cores.
"""
import os
import sys

if "/opt/trn_rl_repo" not in sys.path:
    sys.path.insert(0, "/opt/trn_rl_repo")

import numpy as np
import ml_dtypes

BF16 = ml_dtypes.bfloat16

A, NVIEW, FEAT, BANK, C = 256, 8, 256, 2048, 19
NROWS = A * NVIEW              # 2048 anchor rows
NBLK = C - 1                   # 18 class blocks
NCOLS = NBLK * BANK            # 36864 contrast columns
NCORES = 8
RPC = NROWS // NCORES          # 256 rows per core
G = RPC // 128                 # 2 partition groups per core

_PROGRAM = None
LAST_RESULT = None             # BassKernelResults of the most recent run
RUN_KWARGS = {}                # extra kwargs for run_bass_kernel_spmd (e.g. trace)


def _ensure_ntff_hook():
    """Provide antenv.axon_hooks (NTFF profiling hook) when the image lacks it.

    Replicates trn_agent_boot's ctypes hook against libaxon_pjrt.so so that
    run_bass_kernel_spmd(trace=True) can capture per-core NTFF profiles."""
    import types
    import ctypes
    import contextlib

    try:
        from antenv.axon_hooks import get_axon_ntff_profile_hook  # noqa: F401
        return
    except ImportError:
        pass

    so_path = "/opt/axon/libaxon_pjrt.so"
    if not os.path.exists(so_path):
        return
    try:
        lib = ctypes.CDLL(so_path)
    except OSError:
        return
    if not hasattr(lib, "axon_start_nrt_profile"):
        return
    lib.axon_start_nrt_profile.argtypes = [ctypes.POINTER(ctypes.c_int64),
                                           ctypes.c_size_t]
    lib.axon_start_nrt_profile.restype = ctypes.c_int64
    lib.axon_stop_nrt_profile.argtypes = [ctypes.c_char_p]
    lib.axon_stop_nrt_profile.restype = ctypes.c_int64

    @contextlib.contextmanager
    def _hook(output_dir, device_ids):
        import jax
        jax.devices()
        if device_ids:
            ids = (ctypes.c_int64 * len(device_ids))(*device_ids)
            rc = lib.axon_start_nrt_profile(ids, len(device_ids))
        else:
            rc = lib.axon_start_nrt_profile(None, 0)
        if rc != 0:
            raise RuntimeError(f"axon_start_nrt_profile rc={rc}")
        try:
            yield
        finally:
            n = lib.axon_stop_nrt_profile(str(output_dir).encode())
            print(f"ntff profile: {n} file(s) written to {output_dir}",
                  file=sys.stderr)

    mod = types.ModuleType("antenv.axon_hooks")
    mod.get_axon_ntff_profile_hook = lambda: _hook
    mod.set_axon_ntff_profile_hook = lambda h: None
    sys.modules["antenv.axon_hooks"] = mod


def _build_program():
    from contextlib import ExitStack
    from concourse import bacc, tile, mybir

    dt = mybir.dt
    fp32 = dt.float32
    bf16 = dt.bfloat16
    Act = mybir.ActivationFunctionType
    Alu = mybir.AluOpType

    nc = bacc.Bacc("TRN2", target_bir_lowering=False, debug=False,
                   enable_asserts=False, num_devices=NCORES)

    qt = nc.dram_tensor("qt", [NBLK, 2, 128, 2048], bf16, kind="ExternalInput").ap()
    at = nc.dram_tensor("at", [G, 2, 128, 128], bf16, kind="ExternalInput").ap()
    qx = nc.dram_tensor("qx", [G, 2, 128, 256], bf16, kind="ExternalInput").ap()
    oneh = nc.dram_tensor("oneh", [G, 128, NBLK], fp32, kind="ExternalInput").ap()
    hdv = nc.dram_tensor("hdv", [G, 128, 1], fp32, kind="ExternalInput").ap()
    cntv = nc.dram_tensor("cntv", [G, 128, 1], fp32, kind="ExternalInput").ap()
    nicv = nc.dram_tensor("nicv", [G, 128, 1], fp32, kind="ExternalInput").ap()
    imat = nc.dram_tensor("imat", [128, 128], fp32, kind="ExternalInput").ap()
    lossr = nc.dram_tensor("lossr", [G, 128, 1], fp32, kind="ExternalOutput").ap()

    with tile.TileContext(nc) as tc, ExitStack() as ctx:
        pers = ctx.enter_context(tc.tile_pool(name="pers", bufs=1))
        qtp = ctx.enter_context(tc.tile_pool(name="qtp", bufs=4))
        scr = ctx.enter_context(tc.tile_pool(name="scr", bufs=3))
        vec = ctx.enter_context(tc.tile_pool(name="vec", bufs=1))
        pp = ctx.enter_context(tc.tile_pool(name="pp", bufs=2, space="PSUM"))

        # ---- persistent small tensors -> SBUF
        at_sb = [[pers.tile([128, 128], bf16, name=f"at{g}{k}", tag=f"at{g}{k}") for k in range(2)]
                 for g in range(G)]
        qx_sb = [[pers.tile([128, 256], bf16, name=f"qx{g}{k}", tag=f"qx{g}{k}") for k in range(2)]
                 for g in range(G)]
        oneh_sb = [pers.tile([128, NBLK], fp32, name=f"oneh{g}", tag=f"oneh{g}") for g in range(G)]
        hd_sb = [pers.tile([128, 1], fp32, name=f"hd{g}", tag=f"hd{g}") for g in range(G)]
        cnt_sb = [pers.tile([128, 1], fp32, name=f"cnt{g}", tag=f"cnt{g}") for g in range(G)]
        nic_sb = [pers.tile([128, 1], fp32, name=f"nic{g}", tag=f"nic{g}") for g in range(G)]
        im_sb = pers.tile([128, 128], fp32, name="im", tag="im")
        tbuf = [pers.tile([128, NBLK], fp32, name=f"tbuf{g}", tag=f"tbuf{g}") for g in range(G)]

        nc.sync.dma_start(out=im_sb[:], in_=imat[:])
        for g in range(G):
            for k in range(2):
                nc.sync.dma_start(out=at_sb[g][k][:], in_=at[g, k])
                nc.sync.dma_start(out=qx_sb[g][k][:], in_=qx[g, k])
            nc.sync.dma_start(out=oneh_sb[g][:], in_=oneh[g])
            nc.sync.dma_start(out=hd_sb[g][:], in_=hdv[g])
            nc.sync.dma_start(out=cnt_sb[g][:], in_=cntv[g])
            nc.sync.dma_start(out=nic_sb[g][:], in_=nicv[g])

        # ---- per-row diag + block-sum dots via diagonal of a small matmul
        zd = [vec.tile([128, 1], fp32, name=f"zd{g}", tag=f"zd{g}") for g in range(G)]
        zbs = [vec.tile([128, 1], fp32, name=f"zbs{g}", tag=f"zbs{g}") for g in range(G)]
        for g in range(G):
            psx = pp.tile([128, 2048], fp32, name="ps", tag="ps")
            for k in range(2):
                nc.tensor.matmul(psx[:, 0:256], lhsT=at_sb[g][k][:],
                                 rhs=qx_sb[g][k][:],
                                 start=(k == 0), stop=(k == 1))
            dscr = scr.tile([128, 128], fp32, name="dscr", tag="dscr")
            nc.vector.tensor_tensor(dscr[:], psx[:, 0:128], im_sb[:], op=Alu.mult)
            nc.vector.tensor_reduce(zd[g][:], dscr[:],
                                    axis=mybir.AxisListType.X, op=Alu.add)
            dscr2 = scr.tile([128, 128], fp32, name="dscr", tag="dscr")
            nc.vector.tensor_tensor(dscr2[:], psx[:, 128:256], im_sb[:], op=Alu.mult)
            nc.vector.tensor_reduce(zbs[g][:], dscr2[:],
                                    axis=mybir.AxisListType.X, op=Alu.add)

        # Ed = exp(10*zd) early (same ACT table set as the block exps)
        ed = [vec.tile([128, 1], fp32, name=f"ed{g}", tag=f"ed{g}") for g in range(G)]
        for g in range(G):
            nc.scalar.activation(ed[g][:], zd[g][:], Act.Exp, scale=10.0)

        # ---- phase A: stream the 18 class blocks
        for c in range(NBLK):
            qts = []
            for k in range(2):
                t = qtp.tile([128, 2048], bf16, name=f"qt{k}", tag=f"qt{k}")
                nc.sync.dma_start(out=t[:], in_=qt[c, k])
                qts.append(t)
            for g in range(G):
                ps = pp.tile([128, 2048], fp32, name="ps", tag="ps")
                for k in range(2):
                    for s in range(4):
                        nc.tensor.matmul(ps[:, s * 512:(s + 1) * 512],
                                         lhsT=at_sb[g][k][:],
                                         rhs=qts[k][:, s * 512:(s + 1) * 512],
                                         start=(k == 0), stop=(k == 1))
                so = scr.tile([128, 2048], bf16, name="scr", tag="scr")
                nc.scalar.activation(so[:], ps[:], Act.Exp, scale=10.0,
                                     accum_out=tbuf[g][:, c:c + 1])

        # ---- phase B: assemble per-row losses
        for g in range(G):
            tg = vec.tile([128, 1], fp32, name=f"T{g}", tag=f"T{g}")
            nc.vector.tensor_reduce(tg[:], tbuf[g][:], axis=mybir.AxisListType.X,
                                    op=Alu.add)
            bsc = scr.tile([128, NBLK], fp32, name="bscr", tag="bscr")
            bg = vec.tile([128, 1], fp32, name=f"B{g}", tag=f"B{g}")
            nc.vector.tensor_tensor(bsc[:], tbuf[g][:], oneh_sb[g][:], op=Alu.mult)
            nc.vector.tensor_reduce(bg[:], bsc[:],
                                    axis=mybir.AxisListType.X, op=Alu.add)
            # S = T + BANK - B
            sg = vec.tile([128, 1], fp32, name=f"S{g}", tag=f"S{g}")
            nc.vector.scalar_tensor_tensor(
                out=sg[:], in0=tg[:], scalar=float(BANK), in1=bg[:],
                op0=Alu.add, op1=Alu.subtract)
            lns = vec.tile([128, 1], fp32, name=f"lnS{g}", tag=f"lnS{g}")
            nc.scalar.activation(lns[:], sg[:], Act.Ln)
            rs = vec.tile([128, 1], fp32, name=f"rS{g}", tag=f"rS{g}")
            nc.vector.reciprocal(rs[:], sg[:])

            # pterm = 10*zbs - 10*hd*zd - cnt*lnS - (B - hd*Ed)/S
            t1 = vec.tile([128, 1], fp32, name=f"t1{g}", tag=f"t1{g}")
            nc.vector.tensor_tensor(t1[:], hd_sb[g][:], zd[g][:], op=Alu.mult)
            u = vec.tile([128, 1], fp32, name=f"u{g}", tag=f"u{g}")
            nc.vector.tensor_sub(u[:], zbs[g][:], t1[:])
            v = vec.tile([128, 1], fp32, name=f"v{g}", tag=f"v{g}")
            nc.vector.tensor_tensor(v[:], cnt_sb[g][:], lns[:], op=Alu.mult)
            t2 = vec.tile([128, 1], fp32, name=f"t2{g}", tag=f"t2{g}")
            nc.vector.tensor_tensor(t2[:], hd_sb[g][:], ed[g][:], op=Alu.mult)
            t3 = vec.tile([128, 1], fp32, name=f"t3{g}", tag=f"t3{g}")
            nc.vector.tensor_sub(t3[:], bg[:], t2[:])
            w = vec.tile([128, 1], fp32, name=f"w{g}", tag=f"w{g}")
            nc.vector.tensor_tensor(w[:], t3[:], rs[:], op=Alu.mult)
            p1 = vec.tile([128, 1], fp32, name=f"p1{g}", tag=f"p1{g}")
            nc.vector.scalar_tensor_tensor(
                out=p1[:], in0=u[:], scalar=10.0, in1=v[:],
                op0=Alu.mult, op1=Alu.subtract)
            p2 = vec.tile([128, 1], fp32, name=f"p2{g}", tag=f"p2{g}")
            nc.vector.tensor_sub(p2[:], p1[:], w[:])
            nl = vec.tile([128, 1], fp32, name=f"nl{g}", tag=f"nl{g}")
            nc.vector.tensor_tensor(nl[:], p2[:], nic_sb[g][:], op=Alu.mult)
            nc.sync.dma_start(out=lossr[g], in_=nl[:])

    nc.compile()
    return nc


def _get_program():
    global _PROGRAM
    if _PROGRAM is None:
        _PROGRAM = _build_program()
    return _PROGRAM


def _stage_inputs(X_anchor, y_anchor, queue):
    """Host-side sharding/staging. Returns per-core input maps."""
    X = np.asarray(X_anchor, np.float32)
    y = np.asarray(y_anchor, np.int32)
    Q3 = np.asarray(queue, np.float32)

    AF = X.transpose(1, 0, 2).reshape(NROWS, FEAT)      # view-major rows
    y_rows = np.tile(y, NVIEW)
    perm = np.argsort(y_rows, kind="stable")
    AF_s, y_s, orig_s = AF[perm], y_rows[perm], perm

    Q = Q3[1:].reshape(NCOLS, FEAT)                     # classes 1..18
    QT = np.ascontiguousarray(Q.T)                      # [256, 36864]
    qt = np.ascontiguousarray(
        QT.reshape(2, 128, NBLK, BANK).transpose(2, 0, 1, 3)).astype(BF16)
    qbsum = Q.reshape(NBLK, BANK, FEAT).sum(axis=1, dtype=np.float32)  # [18, 256]
    imat = np.eye(128, dtype=np.float32)

    in_maps = []
    for kcore in range(NCORES):
        rows = slice(kcore * RPC, (kcore + 1) * RPC)
        yk, ok = y_s[rows], orig_s[rows]
        AFk = AF_s[rows]                                # [256, 256]
        ATf = np.ascontiguousarray(AFk.T)               # [feat, row]
        at = np.ascontiguousarray(
            ATf.reshape(2, 128, G, 128).transpose(2, 0, 1, 3)).astype(BF16)

        hd = (yk == 1).astype(np.float32)
        qdiag = np.where(hd[:, None] > 0, Q3[1][ok], 0.0).astype(np.float32)
        qbs = qbsum[yk - 1]                             # [256, 256]
        QD, QB = qdiag.T, qbs.T                         # [feat, row]
        qxa = np.empty((G, 2, 128, 256), np.float32)
        for g in range(G):
            rs = slice(g * 128, (g + 1) * 128)
            blk = np.concatenate([QD[:, rs], QB[:, rs]], axis=1)  # [256, 256]
            qxa[g] = blk.reshape(2, 128, 256)
        qx = qxa.astype(BF16)

        oneh = np.zeros((RPC, NBLK), np.float32)
        oneh[np.arange(RPC), yk - 1] = 1.0
        cnt = (np.float32(BANK) - hd).astype(np.float32)
        nic = (-1.0 / cnt).astype(np.float32)

        in_maps.append({
            "qt": qt,
            "at": at,
            "qx": qx,
            "oneh": np.ascontiguousarray(oneh.reshape(G, 128, NBLK)),
            "hdv": np.ascontiguousarray(hd.reshape(G, 128, 1)),
            "cntv": np.ascontiguousarray(cnt.reshape(G, 128, 1)),
            "nicv": np.ascontiguousarray(nic.reshape(G, 128, 1)),
            "imat": imat,
        })
    return in_maps


def kernel(X_anchor, y_anchor, queue):
    global LAST_RESULT
    _ensure_ntff_hook()
    from concourse.bass_utils import run_bass_kernel_spmd

    nc = _get_program()
    in_maps = _stage_inputs(X_anchor, y_anchor, queue)
    res = run_bass_kernel_spmd(nc, in_maps, list(range(NCORES)), **RUN_KWARGS)
    LAST_RESULT = res
    total = np.float64(0.0)
    for r in res.results:
        total += np.asarray(r["lossr"], np.float64).sum()
    return np.float32(total / NROWS)



# revision 3
# speedup vs baseline: 1.6239x; 1.6239x over previous
"""Trainium2 Bass kernel for ContrastMemoryBankCELoss.

Strategy (8 NeuronCores, SPMD, no collectives) — v2, column-sharded:
  * The 36864 contrast columns (classes 1..18) are sharded across cores:
    core k owns class blocks 2k and 2k+1 (2048 cols each, runs X and Y)
    plus one 512-col quarter of block 16 (k<4) or 17 (k>=4) (run Q).
    All 2048 anchor rows (view-major) are replicated on every core as
    16 groups of 128 partition-rows.
  * Inputs are pre-scaled by sqrt(0.625) and quantized to fp8 e4m3 so a
    single DoubleRow matmul (contraction 256 = 128 partitions x 2
    interleave) yields t = (10/16)*(a.q) in PSUM fp32 at ~216 ns per
    [128,512] tile.
  * Row-wise exp sums are computed by BOTH ScalarE and VectorE in
    parallel:
      - ACT path: activation(Exp, scale=16) with accum_out -> sum exp(u).
      - DVE path: custom fused op EXP16_SQ_ANT computing
        ((1+t)^2+1)^16 = 2^16 * (1+t+t^2/2)^16 ~= 2^16 * exp(16t)
        in one pass from PSUM with accumulate (7 ALU slices + accum).
        Weighted bias of the approximation is ~0.03% (validated 6e-5
        final rel err vs reference).
    A build-time greedy cadence balances the two engines (~47 us each);
    per-(group,run-fragment) partial sums land in per-engine SBUF accum
    tiles and DMA out at the end ([128, 80] fp32 total per core).
  * Everything else is exact fp64 host math: zbs (anchor . class block
    sum), the class-1 diagonal term zd, S = T - B + 2048, and the
    first-order positive-tail formula
      loss_r = -[ zbs - hd*zd - cnt*ln(S) - (B - hd*e^zd)/S ] / cnt,
    which matches the reference to ~2e-7.
"""
import os
import sys

if "/opt/trn_rl_repo" not in sys.path:
    sys.path.insert(0, "/opt/trn_rl_repo")

import numpy as np
import ml_dtypes
from operator import add as _add

FP8 = ml_dtypes.float8_e4m3

A, NVIEW, FEAT, BANK, C = 256, 8, 256, 2048, 19
NROWS = A * NVIEW              # 2048 anchor rows
NBLK = C - 1                   # 18 class blocks
NCOLS = NBLK * BANK            # 36864 contrast columns
NCORES = 8
CPC = NCOLS // NCORES          # 4608 columns per core
NG = NROWS // 128              # 16 row groups
SCALE = float(np.sqrt(0.625))  # joint prescale: t = 0.625 * (a.q) = u/16

# per-group PSUM tile fragments: (slot, size, col_offset_in_core)
SLOT_SIZES = [1536, 512, 1536, 512, 512]
SLOT_OFF = [0, 1536, 2048, 3584, 4096]

# measured sustained per-op engine costs (ns) for cadence balancing
_COST_A = {1536: 1754, 512: 901}
_COST_D = {1536: 1800, 512: 733}

_PROGRAM = None
LAST_RESULT = None
RUN_KWARGS = {}


# --------------------------------------------------------------------------
# custom DVE op registration
# --------------------------------------------------------------------------
def _register_exp16():
    from concourse import dve_ops as _dve_ops
    from concourse.dve_spec import Spec, Src0, C0, Zero, sq

    def _ref_exp16(in0, in1, s0, s1, imm2):
        t = in0.astype(np.float32)
        b = ((s0 + t) * (s0 + t) + s0).astype(np.float32)
        for _ in range(4):
            b = (b * b).astype(np.float32)
        return b, b.reshape(b.shape[0], -1).sum(-1, keepdims=True).astype(np.float32)

    _s = Src0 + C0
    op = _dve_ops.DveOp(
        "EXP16_SQ_ANT",
        Spec(body=sq(sq(sq(sq(sq(_s) + C0)))), accum=_add, accum_init=Zero,
             reference=_ref_exp16),
        subdim=False,
        uops_sha={"v3": "0f695d0542ee22ff", "v4": "594100af0488a0af"},
    )
    for existing in _dve_ops.OPS:
        if existing.name == op.name:
            return existing
    _dve_ops.OPS.append(op)
    _dve_ops.CUSTOM_DVE_SPECS[op.name] = op.spec
    _dve_ops._SUB_OPCODE_FOR_NAME[op.name] = (
        max(_dve_ops._SUB_OPCODE_FOR_NAME.values()) + 1
    )
    return op


def make_assignment():
    """Greedy engine cadence. Returns list over (g, slot) in issue order of
    (engine, accum_col) plus per-engine totals; deterministic on host+build."""
    tA = tD = 0.0
    nA = nD = 0
    out = []
    for g in range(NG):
        for slot, size in enumerate(SLOT_SIZES):
            cA, cD = _COST_A[size], _COST_D[size]
            if tA + cA <= tD + cD:
                out.append(("A", nA))
                tA += cA
                nA += 1
            else:
                out.append(("D", nD))
                tD += cD
                nD += 1
    return out, nA, nD, tA, tD


def _ensure_ntff_hook():
    """Provide antenv.axon_hooks (NTFF profiling hook) when the image lacks it."""
    import types
    import ctypes
    import contextlib

    try:
        from antenv.axon_hooks import get_axon_ntff_profile_hook  # noqa: F401
        return
    except ImportError:
        pass

    so_path = "/opt/axon/libaxon_pjrt.so"
    if not os.path.exists(so_path):
        return
    try:
        lib = ctypes.CDLL(so_path)
    except OSError:
        return
    if not hasattr(lib, "axon_start_nrt_profile"):
        return
    lib.axon_start_nrt_profile.argtypes = [ctypes.POINTER(ctypes.c_int64),
                                           ctypes.c_size_t]
    lib.axon_start_nrt_profile.restype = ctypes.c_int64
    lib.axon_stop_nrt_profile.argtypes = [ctypes.c_char_p]
    lib.axon_stop_nrt_profile.restype = ctypes.c_int64

    @contextlib.contextmanager
    def _hook(output_dir, device_ids):
        import jax
        jax.devices()
        if device_ids:
            ids = (ctypes.c_int64 * len(device_ids))(*device_ids)
            rc = lib.axon_start_nrt_profile(ids, len(device_ids))
        else:
            rc = lib.axon_start_nrt_profile(None, 0)
        if rc != 0:
            raise RuntimeError(f"axon_start_nrt_profile rc={rc}")
        try:
            yield
        finally:
            n = lib.axon_stop_nrt_profile(str(output_dir).encode())
            print(f"ntff profile: {n} file(s) written to {output_dir}",
                  file=sys.stderr)

    mod = types.ModuleType("antenv.axon_hooks")
    mod.get_axon_ntff_profile_hook = lambda: _hook
    mod.set_axon_ntff_profile_hook = lambda h: None
    sys.modules["antenv.axon_hooks"] = mod


def _build_program():
    from contextlib import ExitStack
    from concourse import bacc, tile, mybir

    exp16 = _register_exp16()
    assign, nA, nD, tA, tD = make_assignment()

    dt = mybir.dt
    fp32 = dt.float32
    bf16 = dt.bfloat16
    f8 = dt.float8e4
    Act = mybir.ActivationFunctionType

    nc = bacc.Bacc("TRN2", target_bir_lowering=False, debug=False,
                   enable_asserts=False, num_devices=NCORES)

    at8 = nc.dram_tensor("at8", [128, 2 * NG, 128], f8, kind="ExternalInput").ap()
    qt8 = nc.dram_tensor("qt8", [9, 128, 2, 512], f8, kind="ExternalInput").ap()
    tba = nc.dram_tensor("tba", [128, nA], fp32, kind="ExternalOutput").ap()
    tbd = nc.dram_tensor("tbd", [128, nD], fp32, kind="ExternalOutput").ap()

    with tile.TileContext(nc) as tc, ExitStack() as ctx:
        pers = ctx.enter_context(tc.tile_pool(name="pers", bufs=1))
        pa = ctx.enter_context(tc.tile_pool(name="pa", bufs=2, space="PSUM"))
        pd = ctx.enter_context(tc.tile_pool(name="pd", bufs=2, space="PSUM"))

        at_sb = pers.tile([128, 2 * NG, 128], f8, name="at", tag="at")
        qt_sb = [pers.tile([128, 2, 512], f8, name=f"qt{c}", tag=f"qt{c}")
                 for c in range(9)]
        scrA = pers.tile([128, 1536], bf16, name="scrA", tag="scrA")
        scrD = pers.tile([128, 1536], bf16, name="scrD", tag="scrD")
        tbA = pers.tile([128, nA], fp32, name="tbA", tag="tbA")
        tbD = pers.tile([128, nD], fp32, name="tbD", tag="tbD")
        warm = pers.tile([128, 8], fp32, name="warm", tag="warm")
        warmo = pers.tile([128, 8], bf16, name="warmo", tag="warmo")

        # early tiny activation so the 2.7us Exp table load overlaps the DMAs
        nc.vector.memset(warm[:], 0.0)
        nc.scalar.activation(warmo[:], warm[:], Act.Exp, scale=1.0)

        nc.sync.dma_start(out=at_sb[:], in_=at8[:])
        for c in range(9):
            nc.sync.dma_start(out=qt_sb[c][:], in_=qt8[c])

        DR = mybir.MatmulPerfMode.DoubleRow
        ai = 0
        for g in range(NG):
            lhs = at_sb[:, 2 * g:2 * g + 2, :]
            for slot, size in enumerate(SLOT_SIZES):
                pool = pa if size == 1536 else pd
                tag = "big" if size == 1536 else "small"
                t = pool.tile([128, size], fp32, name=f"t{slot}", tag=tag)
                c0 = SLOT_OFF[slot] // 512
                for j in range(size // 512):
                    nc.tensor.matmul(t[:, j * 512:(j + 1) * 512], lhsT=lhs,
                                     rhs=qt_sb[c0 + j][:], start=True, stop=True,
                                     perf_mode=DR)
                eng, col = assign[ai]
                ai += 1
                if eng == "A":
                    nc.scalar.activation(scrA[:, 0:size], t[:], Act.Exp,
                                         scale=16.0,
                                         accum_out=tbA[:, col:col + 1])
                else:
                    nc.vector._custom_dve(exp16, out=scrD[:, 0:size], in0=t[:],
                                          s0=1.0,
                                          accum_out=tbD[:, col:col + 1])

        nc.sync.dma_start(out=tba[:], in_=tbA[:])
        nc.sync.dma_start(out=tbd[:], in_=tbD[:])

    nc.compile()
    return nc


def _get_program():
    global _PROGRAM
    if _PROGRAM is None:
        _PROGRAM = _build_program()
    return _PROGRAM


def _core_cols(k):
    """Global contrast-column indices owned by core k, in core-local order."""
    x = np.arange(2 * k * BANK, (2 * k + 1) * BANK)
    yy = np.arange((2 * k + 1) * BANK, (2 * k + 2) * BANK)
    qb = 16 if k < 4 else 17
    qq = np.arange(qb * BANK + (k % 4) * 512, qb * BANK + (k % 4) * 512 + 512)
    return np.concatenate([x, yy, qq])


def _stage_inputs(X_anchor, y_anchor, queue):
    X = np.asarray(X_anchor, np.float32)
    Q3 = np.asarray(queue, np.float32)

    AF = X.transpose(1, 0, 2).reshape(NROWS, FEAT)          # view-major rows
    Qm = Q3[1:].reshape(NCOLS, FEAT)                        # classes 1..18

    a8 = (AF * SCALE).astype(FP8)                           # [2048, 256]
    q8 = (Qm * SCALE).astype(FP8)                           # [36864, 256]

    # anchors: at8[p, 2g+i, m] = a8[row=128g+m, feat=128i+p]
    at8 = np.ascontiguousarray(
        a8.reshape(NG, 128, 2, 128).transpose(3, 0, 2, 1).reshape(128, 2 * NG, 128))

    in_maps = []
    for k in range(NCORES):
        cols = _core_cols(k)
        qk = q8[cols]                                       # [4608, 256]
        # qt8[c, p, i, n] = qk[col=512c+n, feat=128i+p]
        qt8 = np.ascontiguousarray(
            qk.reshape(9, 512, 2, 128).transpose(0, 3, 2, 1))
        in_maps.append({"at8": at8, "qt8": qt8})
    return in_maps


def kernel(X_anchor, y_anchor, queue):
    global LAST_RESULT
    _ensure_ntff_hook()
    from concourse.bass_utils import run_bass_kernel_spmd

    nc = _get_program()
    in_maps = _stage_inputs(X_anchor, y_anchor, queue)
    res = run_bass_kernel_spmd(nc, in_maps, list(range(NCORES)), **RUN_KWARGS)
    LAST_RESULT = res

    assign, nA, nD, tA, tD = make_assignment()

    X = np.asarray(X_anchor, np.float64)
    y = np.asarray(y_anchor, np.int64)
    Q3 = np.asarray(queue, np.float64)
    AF = X.transpose(1, 0, 2).reshape(NROWS, FEAT)
    Qm = Q3[1:].reshape(NCOLS, FEAT)
    y_rows = np.tile(y, NVIEW)

    # device partial sums -> per-(core, group, slot) fragment sums
    T = np.zeros(NROWS)
    B = np.zeros(NROWS)
    for k in range(NCORES):
        r = res.results[k]
        va = np.asarray(r["tba"], np.float64)
        vd = np.asarray(r["tbd"], np.float64) / 65536.0
        cls_of_slot = [2 * k + 1, 2 * k + 1, 2 * k + 2, 2 * k + 2,
                       17 if k < 4 else 18]
        ai = 0
        for g in range(NG):
            rows = slice(g * 128, (g + 1) * 128)
            for slot in range(5):
                eng, col = assign[ai]
                ai += 1
                v = va[:, col] if eng == "A" else vd[:, col]
                T[rows] += v
                m = y_rows[rows] == cls_of_slot[slot]
                B[rows.start:rows.stop][m] += v[m]

    # exact host-side terms
    qbsum = Qm.reshape(NBLK, BANK, FEAT).sum(1)
    ZBS = 10.0 * np.einsum('rf,rf->r', AF, qbsum[y_rows - 1])
    hd = (y_rows == 1).astype(np.float64)
    zd = 10.0 * np.einsum('rf,rf->r', AF, Qm[np.arange(NROWS)]) * hd
    Ed = np.exp(zd) * hd

    S = T - B + float(BANK)
    cnt = float(BANK) - hd
    approx = (ZBS - hd * zd) - cnt * np.log(S) - (B - Ed) / S
    loss = float((-(approx / cnt)).mean())
    return np.float32(loss)
